# revision 1
# baseline (speedup 1.0000x reference)
"""Trainium2 Bass kernel for nn_ComplexGraph_19791209300074 (gnn radius-graph
edge construction).

Strategy
--------
Data-parallel over graphs: 64 graphs x 512 nodes; each of the 8 NeuronCores
handles 8 graphs. The device computes, for every same-graph node pair (i, j),
the squared distance d2 = |xi|^2 + |xj|^2 - 2 xi.xj via a single K=5 f32
matmul per 128-row block ([1, -2x, -2y, -2z, |xi|^2]^T . [|xj|^2, x, y, z, 1])
accumulated in PSUM, copied to SBUF by the scalar engine and DMA'd out.

The host then reproduces the reference's edge assembly exactly:
 * the candidate-pair stream is jnp.nonzero(same_bid) in row-major order,
   including an XLA-CPU int32 division quirk: for flat index v >= 2^23 with
   v % 512 == 511 the unraveled (row, col) comes back as (v//512 + 1, -1),
   which turns the pair (r, last-node-of-own-graph) into the cross-graph pair
   (r+1, last-node-of-previous-graph);
 * the distance predicate is the jit-fused f32
   sqrt(fma(dz,dz, fma(dx,dx, dy*dy))) <= cutoff, replicated bitwise on the
   host (float64-emulated fma) for pairs where the device value lies within
   a guard band of a threshold, and for the quirk's cross-graph pairs.

Device d2 is within ~1e-3 of the exact f32 value (measured); the guard band
is 0.05 on d2, so only a few hundred pairs need host refinement.
"""

import numpy as np

NPG = 512          # nodes per graph (hardcoded fast path)
NGRAPH = 64
NCORES = 8
N_TOTAL = NPG * NGRAPH
TWO23 = 2 ** 23
BAND = 0.05        # d2 guard band around thresholds for host refinement

_NC_CACHE = {}
LAST_RESULT = None  # BassKernelResults of the last device run (for test.py)


# ---------------------------------------------------------------- exact math

def _fma_d2(p0, p1):
    """f32 sum-of-squares exactly as the jit-fused jnp.linalg.norm computes
    it on XLA CPU: s = fma(dz,dz, fma(dx,dx, dy*dy)), emulated via float64
    (the f64 intermediate sums round identically to a true f32 fma for these
    magnitudes; validated bitwise against jax on the full input)."""
    d = (p0 - p1).astype(np.float32)
    dx = d[..., 0].astype(np.float64)
    dy = d[..., 1].astype(np.float64)
    dz = d[..., 2].astype(np.float64)
    s1 = (dy * dy).astype(np.float32)
    s2 = (dx * dx + s1.astype(np.float64)).astype(np.float32)
    s3 = (dz * dz + s2.astype(np.float64)).astype(np.float32)
    return s3


def _exact_dist_le(X3, rows, cols, cutoff):
    s = _fma_d2(X3[rows], X3[cols])
    return np.sqrt(s) <= np.float32(cutoff)


def _d2_threshold(cutoff):
    """Largest f32 s with sqrt(s) <= cutoff (sqrt correctly rounded)."""
    c = np.float32(cutoff)
    t = np.float32(c) * np.float32(c)
    # walk up while predicate holds
    while np.sqrt(np.nextafter(t, np.float32(np.inf), dtype=np.float32)) <= c:
        t = np.nextafter(t, np.float32(np.inf), dtype=np.float32)
    while np.sqrt(t, dtype=np.float32) > c:
        t = np.nextafter(t, np.float32(-np.inf), dtype=np.float32)
    return t


# ---------------------------------------------------------------- device part

def _build_nc():
    import concourse.mybir as mybir
    import concourse.tile as tile
    from concourse import bacc

    dt = mybir.dt
    nc = bacc.Bacc()
    ops = nc.dram_tensor("ops", [5, 8192], dt.float32, kind="ExternalInput")
    out = nc.dram_tensor("out", [8, 512, 512], dt.float32, kind="ExternalOutput")

    with tile.TileContext(nc) as tc:
        with (
            tc.tile_pool(name="io", bufs=1) as io_pool,
            tc.tile_pool(name="ps", bufs=4, space="PSUM") as ps_pool,
            tc.tile_pool(name="ob", bufs=4) as out_pool,
        ):
            t = io_pool.tile([5, 8192], dt.float32, tag="ops")
            nc.sync.dma_start(t[:], ops[:])
            for g in range(8):
                o = g * 1024
                for b in range(4):
                    ps = ps_pool.tile([128, 512], dt.float32)
                    nc.tensor.matmul(ps[:], t[:, o + b * 128:o + (b + 1) * 128],
                                     t[:, o + 512:o + 1024])
                    ot = out_pool.tile([128, 512], dt.float32)
                    nc.scalar.copy(ot[:], ps[:])
                    nc.sync.dma_start(out[g, b * 128:(b + 1) * 128, :], ot[:])
    nc.compile()
    return nc


def _get_nc():
    if "nc" not in _NC_CACHE:
        _NC_CACHE["nc"] = _build_nc()
    return _NC_CACHE["nc"]


def _make_operands(X3):
    """X3 [32768,3] f32 -> per-core ops [8, 5, 8192] f32.

    Free dim packs [graph(8) x (lhsT cols 512 | rhs cols 512)]:
      lhsT rows: [1, -2x, -2y, -2z, |x|^2];  rhs rows: [|x|^2, x, y, z, 1].
    """
    x = X3.astype(np.float32)
    nsq = (x[:, 0] * x[:, 0] + x[:, 1] * x[:, 1]) + x[:, 2] * x[:, 2]
    ops = np.empty((64, 5, 1024), np.float32)
    xg = x.reshape(64, 512, 3)
    ng = nsq.reshape(64, 512)
    ops[:, 0, 0:512] = 1.0
    ops[:, 1:4, 0:512] = -2.0 * xg.transpose(0, 2, 1)
    ops[:, 4, 0:512] = ng
    ops[:, 0, 512:1024] = ng
    ops[:, 1:4, 512:1024] = xg.transpose(0, 2, 1)
    ops[:, 4, 512:1024] = 1.0
    return ops.reshape(8, 8, 5, 1024).transpose(0, 2, 1, 3).reshape(8, 5, 8192)


def _run_device(X3):
    """Returns dev d2 cube [64, 512, 512] f32 (approximate, banded use only)."""
    global LAST_RESULT
    from concourse.bass_utils import run_bass_kernel_spmd
    nc = _get_nc()
    ops = _make_operands(X3)
    in_maps = [{"ops": np.ascontiguousarray(ops[c])} for c in range(NCORES)]
    res = run_bass_kernel_spmd(nc, in_maps, core_ids=list(range(NCORES)))
    LAST_RESULT = res
    return np.concatenate([r["out"] for r in res.results], axis=0)


# ------------------------------------------------------------- host assembly

def _build_outputs(X, batch_id, segment_id, is_global, in8_fn, in10_fn):
    """Replicates reference() downstream of the distance computation,
    including the nonzero-unravel quirk. See module docstring."""
    N = batch_id.shape[0]
    nB = int(batch_id[-1]) + 1
    lengths = np.bincount(batch_id, minlength=nB).astype(np.int32)
    offsets = np.concatenate([[0], np.cumsum(lengths)[:-1]]).astype(np.int32)
    local = (np.arange(N, dtype=np.int64) - offsets[batch_id]).astype(np.int32)

    jj = np.broadcast_to(np.arange(NPG, dtype=np.int32), (N, NPG))
    keep = jj != local[:, None]
    row0 = np.repeat(np.arange(N, dtype=np.int64), NPG - 1)
    colj = jj[keep].astype(np.int64)
    flat = row0 * NPG + colj

    bad = (flat >= TWO23) & (colj == NPG - 1)
    row = row0 + bad
    colq = colj - NPG * bad          # -1 at quirk slots
    row_cl = np.minimum(row, N - 1)
    colg = colq + offsets[batch_id[row_cl]]
    colg_w = np.where(colg < 0, colg + N, colg)

    rseg = segment_id[row_cl]
    cseg = segment_id[colg_w]
    rglob = is_global[row_cl]
    cglob = is_global[colg_w]
    ng = ~(rglob | cglob)
    regular = ~bad

    in8 = in8_fn(row_cl, colg_w, regular, colj)
    in10 = in10_fn(row_cl, colg_w, regular, colj)

    row32 = row.astype(np.int32)
    colg32 = colg.astype(np.int32)

    sel1 = (rseg == cseg) & (rseg == 1) & ng
    s = sel1 & in8
    ctx1 = np.stack([row32[s], colg32[s]])

    sel2 = (rseg != cseg) & ng
    ir, ic = row32[sel2], colg32[sel2]
    s = sel2 & in10
    inter = np.stack([row32[s], colg32[s]])
    if inter.shape[1] == 0:
        inter = np.array([[ir[0], ic[0]], [ic[0], ir[0]]], dtype=inter.dtype)

    red_sel = inter[0] < inter[1]
    red_bid = batch_id[inter[0][red_sel]]
    red_off = offsets[red_bid]

    selg = (rseg == cseg) & ~ng
    gn = np.stack([row32[selg], colg32[selg]])
    selgg = rglob & cglob
    gg = np.stack([row32[selgg], colg32[selgg]])
    ctx = np.concatenate([ctx1, gn, gg], axis=1)

    return (ctx.astype(np.int32), inter.astype(np.int32),
            red_bid.astype(np.int32), red_off.astype(np.int32))


def _classifier(X3, dev, cutoff):
    """in-predicate over the candidate stream using the device d2 cube with
    banded host refinement. dev: [64, 512, 512] f32."""
    thr = _d2_threshold(cutoff)
    lo = np.float32(thr - BAND)
    hi = np.float32(thr + BAND)
    dev_flat = dev.reshape(-1)

    def fn(row, colg, regular, colj):
        out = np.empty(row.shape[0], dtype=bool)
        r = row[regular]
        # same-graph pair: cube index = g*512*512 + local(r)*512 + colj
        g = r // NPG
        idx = (g * NPG + (r - g * NPG)) * NPG + colj[regular]
        v = dev_flat[idx]
        res = v <= lo
        unc = (v > lo) & (v <= hi)
        if unc.any():
            res[unc] = _exact_dist_le(X3, r[unc], colg[regular][unc], cutoff)
        out[regular] = res
        nb = ~regular
        if nb.any():
            out[nb] = _exact_dist_le(X3, row[nb], colg[nb], cutoff)
        return out

    return fn


# ------------------------------------------------------------------ fallback

def _fallback(X, batch_id, segment_id, is_global):
    """Clean numpy replication (no device, no quirk emulation) for inputs
    that don't match the expected uniform 64x512 structure."""
    X3 = X[:, 0, :]
    N = batch_id.shape[0]
    nB = int(batch_id[-1]) + 1
    lengths = np.bincount(batch_id, minlength=nB).astype(np.int32)
    offsets = np.concatenate([[0], np.cumsum(lengths)[:-1]]).astype(np.int32)
    max_n = int(lengths.max())

    rows, cols = [], []
    for g in range(nB):
        o, L = int(offsets[g]), int(lengths[g])
        r = np.arange(o, o + L)
        rr = np.repeat(r, L)
        cc = np.tile(r, L)
        m = rr != cc
        rows.append(rr[m])
        cols.append(cc[m])
    row = np.concatenate(rows)
    colg = np.concatenate(cols)

    rseg, cseg = segment_id[row], segment_id[colg]
    rglob, cglob = is_global[row], is_global[colg]
    ng = ~(rglob | cglob)
    in8 = _exact_dist_le(X3, row, colg, 8.0)
    in10 = _exact_dist_le(X3, row, colg, 10.0)

    row32, colg32 = row.astype(np.int32), colg.astype(np.int32)
    sel1 = (rseg == cseg) & (rseg == 1) & ng
    ctx1 = np.stack([row32[sel1 & in8], colg32[sel1 & in8]])
    sel2 = (rseg != cseg) & ng
    ir, ic = row32[sel2], colg32[sel2]
    s = sel2 & in10
    inter = np.stack([row32[s], colg32[s]])
    if inter.shape[1] == 0:
        inter = np.array([[ir[0], ic[0]], [ic[0], ir[0]]], dtype=inter.dtype)
    red_sel = inter[0] < inter[1]
    red_bid = batch_id[inter[0][red_sel]]
    red_off = offsets[red_bid]
    selg = (rseg == cseg) & ~ng
    gn = np.stack([row32[selg], colg32[selg]])
    selgg = rglob & cglob
    gg = np.stack([row32[selgg], colg32[selgg]])
    ctx = np.concatenate([ctx1, gn, gg], axis=1)
    return (ctx.astype(np.int32), inter.astype(np.int32),
            red_bid.astype(np.int32), red_off.astype(np.int32))


# ---------------------------------------------------------------------- main

def kernel(X, batch_id, segment_id, is_global):
    X = np.asarray(X, dtype=np.float32)
    batch_id = np.asarray(batch_id, dtype=np.int32)
    segment_id = np.asarray(segment_id, dtype=np.int32)
    is_global = np.asarray(is_global).astype(bool)

    uniform = (batch_id.shape[0] == N_TOTAL and
               np.array_equal(batch_id,
                              np.repeat(np.arange(NGRAPH, dtype=np.int32), NPG)))
    if not uniform:
        return _fallback(X, batch_id, segment_id, is_global)

    X3 = X[:, 0, :]
    dev = _run_device(X3)
    return _build_outputs(X, batch_id, segment_id, is_global,
                          _classifier(X3, dev, 8.0),
                          _classifier(X3, dev, 10.0))


# revision 3
# speedup vs baseline: 3.0953x; 3.0953x over previous
"""Trainium2 Bass kernel for nn_ComplexGraph_19791209300074 (gnn radius-graph
edge construction).

Strategy
--------
Data-parallel over graphs: 64 graphs x 512 nodes; each of the 8 NeuronCores
handles 8 graphs. The device computes, for every same-graph node pair (i, j),
the squared distance d2 = |xi|^2 + |xj|^2 - 2 xi.xj via a single K=5 f32
matmul per 128-row block ([1, -2x, -2y, -2z, |xi|^2]^T . [|xj|^2, x, y, z, 1])
accumulated in PSUM, copied to SBUF by the scalar engine and DMA'd out.

The host then reproduces the reference's edge assembly exactly:
 * the candidate-pair stream is jnp.nonzero(same_bid) in row-major order,
   including an XLA-CPU int32 division quirk: for flat index v >= 2^23 with
   v % 512 == 511 the unraveled (row, col) comes back as (v//512 + 1, -1),
   which turns the pair (r, last-node-of-own-graph) into the cross-graph pair
   (r+1, last-node-of-previous-graph);
 * the distance predicate is the jit-fused f32
   sqrt(fma(dz,dz, fma(dx,dx, dy*dy))) <= cutoff, replicated bitwise on the
   host (float64-emulated fma) for pairs where the device value lies within
   a guard band of a threshold, and for the quirk's cross-graph pairs.

Device d2 is within ~1e-3 of the exact f32 value (measured); the guard band
is 0.05 on d2, so only a few hundred pairs need host refinement.
"""

import numpy as np

NPG = 512          # nodes per graph (hardcoded fast path)
NGRAPH = 64
NCORES = 8
N_TOTAL = NPG * NGRAPH
TWO23 = 2 ** 23
BAND = 5.0         # d2 guard band (fp8 quant max 4.0008 measured near thresholds)

PSUM_OFF = (0, 512, 1024, 1280)   # block b offset in PSUM (banks 0,1,2,2)
PACK_OFF = (0, 512, 896, 1152)    # block b offset in the packed output
WIDTH = (512, 384, 256, 128)
PACKW = 1280

_NC_CACHE = {}
LAST_RESULT = None  # BassKernelResults of the last device run (for test.py)


# ---------------------------------------------------------------- exact math

def _fma_d2(p0, p1):
    """f32 sum-of-squares exactly as the jit-fused jnp.linalg.norm computes
    it on XLA CPU: s = fma(dz,dz, fma(dx,dx, dy*dy)), emulated via float64
    (the f64 intermediate sums round identically to a true f32 fma for these
    magnitudes; validated bitwise against jax on the full input)."""
    d = (p0 - p1).astype(np.float32)
    dx = d[..., 0].astype(np.float64)
    dy = d[..., 1].astype(np.float64)
    dz = d[..., 2].astype(np.float64)
    s1 = (dy * dy).astype(np.float32)
    s2 = (dx * dx + s1.astype(np.float64)).astype(np.float32)
    s3 = (dz * dz + s2.astype(np.float64)).astype(np.float32)
    return s3


def _exact_dist_le(X3, rows, cols, cutoff):
    s = _fma_d2(X3[rows], X3[cols])
    return np.sqrt(s) <= np.float32(cutoff)


def _d2_threshold(cutoff):
    """Largest f32 s with sqrt(s) <= cutoff (sqrt correctly rounded)."""
    c = np.float32(cutoff)
    t = np.float32(c) * np.float32(c)
    # walk up while predicate holds
    while np.sqrt(np.nextafter(t, np.float32(np.inf), dtype=np.float32)) <= c:
        t = np.nextafter(t, np.float32(np.inf), dtype=np.float32)
    while np.sqrt(t, dtype=np.float32) > c:
        t = np.nextafter(t, np.float32(-np.inf), dtype=np.float32)
    return t


# ---------------------------------------------------------------- device part

def _build_nc():
    import concourse.mybir as mybir
    import concourse.tile as tile
    from concourse import bacc

    dt = mybir.dt
    nc = bacc.Bacc()
    ops = nc.dram_tensor("ops", [8, 128, 640], dt.float16, kind="ExternalInput")
    out = nc.dram_tensor("out", [8, 128, PACKW], dt.float8e4, kind="ExternalOutput")

    with tile.TileContext(nc) as tc:
        with (
            tc.tile_pool(name="io", bufs=8) as io_pool,
            tc.tile_pool(name="psA", bufs=2, space="PSUM") as psA_pool,
            tc.tile_pool(name="psB", bufs=2, space="PSUM") as psB_pool,
            tc.tile_pool(name="ob", bufs=8) as out_pool,
        ):
            for g in range(8):
                t = io_pool.tile([128, 640], dt.float16, tag="ops")
                nc.gpsimd.dma_start(t[:], ops[g])
                psA = psA_pool.tile([128, 1024], dt.float32)  # b0 [0:512], b1 [512:896]
                psB = psB_pool.tile([128, 1024], dt.float32)  # b2 [0:256], b3 [512:640]
                # 4 row-group-packed concurrent matmuls (K=13 in 32-row strips)
                nc.tensor.matmul(psA[:, 0:512], t[0:13, 0:128],
                                 t[0:13, 128:640], tile_position=(0, 0))
                nc.tensor.matmul(psA[:, 512:896], t[32:45, 0:128],
                                 t[32:45, 256:640], tile_position=(32, 0))
                nc.tensor.matmul(psB[:, 0:256], t[64:77, 0:128],
                                 t[64:77, 384:640], tile_position=(64, 0))
                nc.tensor.matmul(psB[:, 512:640], t[96:109, 0:128],
                                 t[96:109, 512:640], tile_position=(96, 0))
                ot = out_pool.tile([128, PACKW], dt.float8e4)
                nc.scalar.mul(ot[:, 0:896], psA[:, 0:896], 1.0 / 64.0)
                nc.vector.tensor_scalar_mul(ot[:, 896:1152], psB[:, 0:256],
                                            1.0 / 64.0)
                nc.vector.tensor_scalar_mul(ot[:, 1152:1280], psB[:, 512:640],
                                            1.0 / 64.0)
                nc.sync.dma_start(out[g], ot[:])
    nc.compile()
    return nc


def _get_nc():
    if "nc" not in _NC_CACHE:
        _NC_CACHE["nc"] = _build_nc()
    return _NC_CACHE["nc"]


def _split16(v):
    h = v.astype(np.float16)
    l = (v - h.astype(np.float32)).astype(np.float16)
    return h, l


def _make_operands(X3):
    """X3 [32768,3] f32 -> per-core ops [8, 13, 8192] fp16.

    free dim: g*1024 + [0:512 lhsT cols | 512:1024 rhs cols]
    rows (lhsT | rhs):
      0..8   cross terms, coord c in {x,y,z} rows 3c..3c+2:
             (Ah_c | Bh_c), (Ah_c | Bl_c), (Al_c | Bh_c)   A=-2x, B=x
      9,10   (1 | Nh), (1 | Nl)          N = |x|^2
      11,12  (Mh | 1), (Ml | 1)          M = |x|^2
    """
    x = X3.astype(np.float32)
    nsq = (x[:, 0] * x[:, 0] + x[:, 1] * x[:, 1]) + x[:, 2] * x[:, 2]
    xg = x.reshape(64, 512, 3)
    ng = nsq.reshape(64, 512)
    A = -2.0 * xg                      # [64,512,3] f32 (exact)
    Ah, Al = _split16(A)
    Bh, Bl = _split16(xg)
    Nh, Nl = _split16(ng)

    ops = np.zeros((64, 13, 1024), np.float16)
    for c in range(3):
        ops[:, 3 * c + 0, 0:512] = Ah[:, :, c]
        ops[:, 3 * c + 0, 512:1024] = Bh[:, :, c]
        ops[:, 3 * c + 1, 0:512] = Ah[:, :, c]
        ops[:, 3 * c + 1, 512:1024] = Bl[:, :, c]
        ops[:, 3 * c + 2, 0:512] = Al[:, :, c]
        ops[:, 3 * c + 2, 512:1024] = Bh[:, :, c]
    ops[:, 9, 0:512] = 1.0
    ops[:, 9, 512:1024] = Nh
    ops[:, 10, 0:512] = 1.0
    ops[:, 10, 512:1024] = Nl
    ops[:, 11, 0:512] = Nh
    ops[:, 11, 512:1024] = 1.0
    ops[:, 12, 0:512] = Nl
    ops[:, 12, 512:1024] = 1.0
    # replicate into 4 row-group strips: [64, 128, 640]
    strip = np.zeros((64, 128, 640), np.float16)
    for b in range(4):
        strip[:, 32 * b:32 * b + 13, 0:128] = ops[:, :, 128 * b:128 * (b + 1)]
        strip[:, 32 * b:32 * b + 13, 128:640] = ops[:, :, 512:1024]
    return strip.reshape(8, 8, 128, 640)


def _run_device(X3):
    """Returns decoded d2 values [64, 128, 1280] f32 (triangle-packed)."""
    global LAST_RESULT
    import ml_dtypes
    from concourse.bass_utils import run_bass_kernel_spmd
    nc = _get_nc()
    ops = _make_operands(X3)
    in_maps = [{"ops": np.ascontiguousarray(ops[c])} for c in range(NCORES)]
    res = run_bass_kernel_spmd(nc, in_maps, core_ids=list(range(NCORES)))
    LAST_RESULT = res
    raw = np.concatenate([r["out"] for r in res.results], axis=0)
    b = raw.view(np.uint8) if raw.dtype != np.uint8 else raw
    lut = (np.arange(256, dtype=np.uint8).view(ml_dtypes.float8_e4m3)
           .astype(np.float32) * 64.0)
    return lut[b]


# ------------------------------------------------------------- host assembly

def _build_outputs(X, batch_id, segment_id, is_global, in8_fn, in10_fn):
    """Replicates reference() downstream of the distance computation,
    including the nonzero-unravel quirk. See module docstring."""
    N = batch_id.shape[0]
    nB = int(batch_id[-1]) + 1
    lengths = np.bincount(batch_id, minlength=nB).astype(np.int32)
    offsets = np.concatenate([[0], np.cumsum(lengths)[:-1]]).astype(np.int32)
    local = (np.arange(N, dtype=np.int64) - offsets[batch_id]).astype(np.int32)

    jj = np.broadcast_to(np.arange(NPG, dtype=np.int32), (N, NPG))
    keep = jj != local[:, None]
    row0 = np.repeat(np.arange(N, dtype=np.int64), NPG - 1)
    colj = jj[keep].astype(np.int64)
    flat = row0 * NPG + colj

    bad = (flat >= TWO23) & (colj == NPG - 1)
    row = row0 + bad
    colq = colj - NPG * bad          # -1 at quirk slots
    row_cl = np.minimum(row, N - 1)
    colg = colq + offsets[batch_id[row_cl]]
    colg_w = np.where(colg < 0, colg + N, colg)

    rseg = segment_id[row_cl]
    cseg = segment_id[colg_w]
    rglob = is_global[row_cl]
    cglob = is_global[colg_w]
    ng = ~(rglob | cglob)
    regular = ~bad

    in8 = in8_fn(row_cl, colg_w, regular, colj)
    in10 = in10_fn(row_cl, colg_w, regular, colj)

    row32 = row.astype(np.int32)
    colg32 = colg.astype(np.int32)

    sel1 = (rseg == cseg) & (rseg == 1) & ng
    s = sel1 & in8
    ctx1 = np.stack([row32[s], colg32[s]])

    sel2 = (rseg != cseg) & ng
    ir, ic = row32[sel2], colg32[sel2]
    s = sel2 & in10
    inter = np.stack([row32[s], colg32[s]])
    if inter.shape[1] == 0:
        inter = np.array([[ir[0], ic[0]], [ic[0], ir[0]]], dtype=inter.dtype)

    red_sel = inter[0] < inter[1]
    red_bid = batch_id[inter[0][red_sel]]
    red_off = offsets[red_bid]

    selg = (rseg == cseg) & ~ng
    gn = np.stack([row32[selg], colg32[selg]])
    selgg = rglob & cglob
    gg = np.stack([row32[selgg], colg32[selgg]])
    ctx = np.concatenate([ctx1, gn, gg], axis=1)

    return (ctx.astype(np.int32), inter.astype(np.int32),
            red_bid.astype(np.int32), red_off.astype(np.int32))


def _make_classifiers(X3, dec):
    """Returns (in8_fn, in10_fn) over the candidate stream.

    dec: [64, 128, 1280] f32 decoded device d2 (upper-triangle packed).
    The gathered per-pair value is computed once and shared.
    """
    dec_flat = dec.reshape(-1)
    pack_off = np.array(PACK_OFF, dtype=np.int64)
    cache = {}

    def gather(row, colj, regular):
        key = "v"
        if key in cache:
            return cache[key]
        r = row[regular]
        g = r // NPG
        i = r - g * NPG
        j = colj[regular]
        ib = i // 128
        jb = j // 128
        swap = jb < ib
        a = np.where(swap, j, i)
        b = np.where(swap, i, j)
        ab = np.where(swap, jb, ib)
        idx = (g * 128 + (a - ab * 128)) * PACKW + pack_off[ab] + (b - ab * 128)
        v = dec_flat[idx]
        cache[key] = v
        return v

    def mk(cutoff):
        thr = float(_d2_threshold(cutoff))
        lo = np.float32(thr - BAND)
        hi = np.float32(thr + BAND)

        def fn(row, colg, regular, colj):
            out = np.empty(row.shape[0], dtype=bool)
            v = gather(row, colj, regular)
            res = v <= lo
            unc = (v > lo) & (v <= hi)
            if unc.any():
                r = row[regular]
                res[unc] = _exact_dist_le(X3, r[unc], colg[regular][unc], cutoff)
            out[regular] = res
            nb = ~regular
            if nb.any():
                out[nb] = _exact_dist_le(X3, row[nb], colg[nb], cutoff)
            return out

        return fn

    return mk(8.0), mk(10.0)


# ------------------------------------------------------------------ fallback

def _fallback(X, batch_id, segment_id, is_global):
    """Clean numpy replication (no device, no quirk emulation) for inputs
    that don't match the expected uniform 64x512 structure."""
    X3 = X[:, 0, :]
    N = batch_id.shape[0]
    nB = int(batch_id[-1]) + 1
    lengths = np.bincount(batch_id, minlength=nB).astype(np.int32)
    offsets = np.concatenate([[0], np.cumsum(lengths)[:-1]]).astype(np.int32)
    max_n = int(lengths.max())

    rows, cols = [], []
    for g in range(nB):
        o, L = int(offsets[g]), int(lengths[g])
        r = np.arange(o, o + L)
        rr = np.repeat(r, L)
        cc = np.tile(r, L)
        m = rr != cc
        rows.append(rr[m])
        cols.append(cc[m])
    row = np.concatenate(rows)
    colg = np.concatenate(cols)

    rseg, cseg = segment_id[row], segment_id[colg]
    rglob, cglob = is_global[row], is_global[colg]
    ng = ~(rglob | cglob)
    in8 = _exact_dist_le(X3, row, colg, 8.0)
    in10 = _exact_dist_le(X3, row, colg, 10.0)

    row32, colg32 = row.astype(np.int32), colg.astype(np.int32)
    sel1 = (rseg == cseg) & (rseg == 1) & ng
    ctx1 = np.stack([row32[sel1 & in8], colg32[sel1 & in8]])
    sel2 = (rseg != cseg) & ng
    ir, ic = row32[sel2], colg32[sel2]
    s = sel2 & in10
    inter = np.stack([row32[s], colg32[s]])
    if inter.shape[1] == 0:
        inter = np.array([[ir[0], ic[0]], [ic[0], ir[0]]], dtype=inter.dtype)
    red_sel = inter[0] < inter[1]
    red_bid = batch_id[inter[0][red_sel]]
    red_off = offsets[red_bid]
    selg = (rseg == cseg) & ~ng
    gn = np.stack([row32[selg], colg32[selg]])
    selgg = rglob & cglob
    gg = np.stack([row32[selgg], colg32[selgg]])
    ctx = np.concatenate([ctx1, gn, gg], axis=1)
    return (ctx.astype(np.int32), inter.astype(np.int32),
            red_bid.astype(np.int32), red_off.astype(np.int32))


# ---------------------------------------------------------------------- main

def kernel(X, batch_id, segment_id, is_global):
    X = np.asarray(X, dtype=np.float32)
    batch_id = np.asarray(batch_id, dtype=np.int32)
    segment_id = np.asarray(segment_id, dtype=np.int32)
    is_global = np.asarray(is_global).astype(bool)

    uniform = (batch_id.shape[0] == N_TOTAL and
               np.array_equal(batch_id,
                              np.repeat(np.arange(NGRAPH, dtype=np.int32), NPG)))
    if not uniform:
        return _fallback(X, batch_id, segment_id, is_global)

    X3 = X[:, 0, :]
    dec = _run_device(X3)
    in8_fn, in10_fn = _make_classifiers(X3, dec)
    return _build_outputs(X, batch_id, segment_id, is_global, in8_fn, in10_fn)
